# revision 23
# baseline (speedup 1.0000x reference)
"""Trainium2 Bass kernel for nn_FPSWE_pool (segment_reduce / sliced-Wasserstein pool).

Math (validated against the reference to ~4e-7 rel):
    W  = theta_v / ||theta_v||_row                       # [Pd, d_in]
    Xs = X @ W.T                                         # [N, Pd]
    S[e, :, p] = sort(Xs[e*32:(e+1)*32, p])              # per-edge, per-col sort
    out[e, p]  = c[p] - (1/M) * sum_r A[r, p] * S[e, r, p]
where A [32, Pd] and c [Pd] are small matrices computed on the host from
(weight, anchors, deg) only: A folds the anchor-grid linear interpolation,
the per-column argsort of anchors, and the weight matrix; c is the
edge-independent term (weight * anchors.T).mean(axis=1).

Sharding: edges are split 8 ways (contiguous 32-row degree blocks), per the
contiguous-block structure of hyperedge_index; params replicated.

Device work per core (256 edges = 8192 rows/core):
  1. input DMA: consts on their own queue first (per-queue DMA bandwidth, not
     HBM, bounds the head), then the bf16 X.T slice as 8 range-chunks so the
     first matmuls start as soon as ~260KB land
  2. bf16 TensorE matmul: Xs.T[ph, rows] = W_h @ X.T (2 proj halves of 128),
     paired into 2-bank PSUM tiles; ScalarE copies cast PSUM fp32 -> SBUF
     bf16 (the first pair as two half copies so sorting starts earlier)
  3. flip-form bitonic sort network (15 stages) over each 32-row block along
     the free dim on VectorE.  Both proj halves live in ONE [128, 16384]
     tile so each stage is a single instruction pair, in 2 chunks (64 + 448
     edges) so the first chunk sorts while the input still streams in.  A
     rotate-by-1 physical slot layout keeps every stage's innermost AP step
     at +-1 so the DVE bf16 2x perf mode applies (proven minimal: one
     flip-32 stage must run strided at 1x -- the slot-bit-0 wire is
     invariant across stages, and every wire bit eventually differs).
  4. A-weighted reduce per half: DVE mult (2x, broadcast A) + pairwise-add
     levels 16/8/4 (2x) + tensor_reduce of 4 (1x; tensor_reduce never
     engages 2x on HW) -> fp32, then the Scalar engine applies the final
     affine (Identity, scale=-1/M, bias=c) off the DVE critical path and
     the output DMA fires per half.

Timing model (all DVE ops verified on HW to ~1%): DVE 0.96 GHz, 2 elem/
cycle/lane in 2x mode, ~156ns per-instruction SBUF-access overhead.  The
kernel is wall-to-wall VectorE (~165us busy, zero gaps); head ~13us is
framework preamble + first DMA chunk + 2 copies; tail ~5us is affine + DMA +
engine drains.  NOTE: the board clock is run-to-run bimodal (~183us at full
clock, ~219us when the firmware throttles DVE to ~0.80 GHz); per-op timing
against the model tells the modes apart.

Compiler notes: walrus allows ONE sync-wait command per compute instruction,
hence the bacc.Bacc + nc.compile() path (event-semaphore conversion) and the
"gate" instructions that pre-absorb DMA waits on PE/DVE/ACT.
"""

import os
from contextlib import ExitStack

import numpy as np

E_EDGES, DEG, D_IN, N_PROJ, M_ANCH = 2048, 32, 128, 256, 128
N_CORES = 8
E_LOC = E_EDGES // N_CORES          # 256 edges per core
ROWS_LOC = E_LOC * DEG              # 8192 rows per core
PH = N_PROJ // 128                  # 2 projection halves
# consts (f32 cols): wt bf16-packed(128) | A bf16-packed(32) | c (2)
W_COLS = N_PROJ // 2                # 128 f32 cols of packed bf16 weights
AB_COLS = PH * DEG // 2             # 32 f32 cols of packed bf16 A (DVE mult)
CONST_W = W_COLS + AB_COLS + PH
X_COLS = ROWS_LOC // 2              # X bf16 packed as f32 cols
IN_W = CONST_W + X_COLS

LAST_RESULTS = None                 # test.py reads trace info from here


# ----------------------------------------------------------------- network ---
def _rot(i):
    return ((i << 1) | (i >> 4)) & 31


def _sort_stages(E):
    """Flip-form bitonic(32) stages in rotate-1 physical layout.

    Returns [(lo_off, lo_dims, hi_off, hi_dims)] over a free axis of E*32
    elements; each side covers E*16 elements, pairing elementwise in stream
    order; ascending logical order (min -> lo side).
    """
    out = []
    for m in range(1, 6):
        if m <= 4:
            lo_dims = [(1 << (m + 1), E * (1 << (4 - m)))]
            hi_dims = [(1 << (m + 1), E * (1 << (4 - m)))]
            if m >= 2:
                lo_dims.append((2, 1 << (m - 1)))
                hi_dims.append((-2, 1 << (m - 1)))
            lo_dims.append((1, 2))
            hi_dims.append((1, 2))
            out.append((0, lo_dims, (1 << (m + 1)) - 2, hi_dims))
        else:
            out.append((0, [(32, E), (2, 16)], 31, [(32, E), (-2, 16)]))
        d = (1 << m) // 4
        while d >= 1:
            f = d.bit_length()          # phys bit = k+1
            lo_dims = [(1 << (f + 1), E * (1 << (4 - f))), (1, 1 << f)]
            hi_dims = [(1 << (f + 1), E * (1 << (4 - f))), (1, 1 << f)]
            out.append((0, lo_dims, 1 << f, hi_dims))
            d //= 2
    assert len(out) == 15
    return out


# ------------------------------------------------------------- bass program ---
def _emit(tc, in_d, o_d):
    """Emit the per-core program.

    in_d [128, IN_W] f32: [ wt bf16-packed | A bf16-packed | c | X.T bf16-packed ]
    o_d  [PH, 128, E_LOC] f32: out.T per proj half
    """
    import concourse.mybir as mybir
    from concourse.ap import AP

    nc = tc.nc
    f32 = mybir.dt.float32
    bf16 = mybir.dt.bfloat16
    CH = 512                     # bf16 row-chunk per matmul
    NCH = ROWS_LOC // CH         # 16

    with ExitStack() as ctx:
        main_p = ctx.enter_context(tc.tile_pool(name="main", bufs=1))
        ps_mm = ctx.enter_context(tc.tile_pool(name="ps_mm", bufs=3, space="PSUM"))
        sort_p = ctx.enter_context(tc.tile_pool(name="sort", bufs=1))
        out_p = ctx.enter_context(tc.tile_pool(name="out", bufs=2))

        xin = main_p.tile([128, IN_W], f32)
        # Consts go first on their own queue so the DVE/ACT consts gates
        # open early; X follows in 8 range-chunks (per-queue DMA bandwidth,
        # not HBM, bounds the head).
        XR = [0, 256, 512, 1024, 1536, 2048, 2560, 3072, 3584, X_COLS]
        MM_GRP = [0, 1, 2, 2, 3, 3, 4, 4, 5, 5, 6, 6, 7, 7, 8, 8]
        nc.sync.dma_start(xin[:, :CONST_W], in_d[:, :CONST_W])
        for gi in range(len(XR) - 1):
            lo, hi = CONST_W + XR[gi], CONST_W + XR[gi + 1]
            nc.sync.dma_start(xin[:, lo:hi], in_d[:, lo:hi])

        wt_sb = xin[:, :W_COLS].bitcast(bf16)                # [128, 256]
        a_sb = xin[:, W_COLS:W_COLS + AB_COLS].bitcast(bf16)  # [128, 64] bf16
        c_sb = xin[:, CONST_W - PH:CONST_W]
        xt = xin[:, CONST_W:].bitcast(bf16)                  # [128, ROWS_LOC]

        # B holds Xs.T bf16 for both proj halves back to back ([h0 | h1]) so
        # one sort-stage instruction can span both halves; C is the sort
        # ping-pong scratch.
        B = sort_p.tile([128, PH * ROWS_LOC], bf16, tag="B", name="B")
        C = sort_p.tile([128, PH * ROWS_LOC], bf16, tag="C", name="C")

        # Walrus allows only ONE sync-wait command on a Matmult (LDW struct).
        # Matmuls at DMA-group seams would need two (new DMA range + PSUM
        # slot release), so a tiny "gate" matmul absorbs each group's DMA
        # wait first; the real matmuls then only wait on the ACT copy.
        from concourse.bass import _add_dep_helper
        ps_gate = ctx.enter_context(tc.tile_pool(name="ps_gate", bufs=1, space="PSUM"))
        gates = {}

        def emit_gate(g):
            pg = ps_gate.tile([128, 1], f32, tag="g", name=f"gate{g}")
            gates[g] = nc.tensor.matmul(
                pg[:], wt_sb[:, :128],
                xt[:, 2 * XR[g]:2 * XR[g] + 1],
                start=True, stop=True,
            )

        # Gates absorbing the consts-range DMA wait on DVE (the A-multiply
        # reads a_sb) and on ACT (the output affine reads c_sb).
        dve_gate_t = out_p.tile([128, 2], f32, name="dve_gate_t")
        dve_gate = nc.vector.tensor_copy(dve_gate_t[:], c_sb[:, :PH])
        act_gate_t = out_p.tile([128, 2], f32, name="act_gate_t")
        nc.scalar.copy(act_gate_t[:], c_sb[:, :PH])

        # HAM warm-up: the PE clock gate needs ~3.4us of sustained activity to
        # lift the cold 1.2 GHz throttle.  Dummy matmuls on a zeroed scratch
        # tile during the preamble+DMA window make the first real (critical-
        # path) matmuls run at 2.4 GHz.
        warm_t = out_p.tile([128, 128], bf16, name="warm_t")
        nc.vector.memset(warm_t[:], 0.0)
        pw = ps_gate.tile([128, 128], f32, tag="w", name="warm_ps")
        for _ in range(12):
            nc.tensor.matmul(pw[:], warm_t[:], warm_t[:], start=True, stop=True)

        for h in range(PH):
            for jj in range(NCH // 2):
                # the very first pair is copied in two halves so the first
                # sort instruction can start after matmul chunk j=1 lands
                # instead of waiting for the full paired copy
                split_copy = h == 0 and jj == 0
                pmm = ps_mm.tile([128, 2 * CH], f32, tag="mm", name=f"mm{h}_{jj}")
                for k in range(2):
                    j = 2 * jj + k
                    g = MM_GRP[j]
                    if g >= 1 and g not in gates:
                        emit_gate(g)
                    mm = nc.tensor.matmul(
                        pmm[:, k * CH:(k + 1) * CH],
                        wt_sb[:, h * 128:(h + 1) * 128],
                        xt[:, j * CH:(j + 1) * CH],
                        start=True, stop=True,
                    )
                    if g >= 1:
                        _add_dep_helper(
                            mm.ins, gates[g].ins, sync=False,
                            reason="order mm after its DMA-group gate",
                        )
                    if split_copy:
                        nc.scalar.copy(
                            B[:, j * CH:(j + 1) * CH],
                            pmm[:, k * CH:(k + 1) * CH])
                if not split_copy:
                    nc.scalar.copy(
                        B[:, h * ROWS_LOC + jj * 2 * CH:
                           h * ROWS_LOC + (jj + 1) * 2 * CH], pmm[:]
                    )

        # Sort chunks in global edge space (h0 edges then h1 edges): a small
        # first chunk starts sorting while the input still streams in; the
        # rest is one chunk so per-instruction dispatch overhead is paid once
        # per stage.  Finishes are per (half, edge-range) since A/c differ
        # between halves.
        SORT_CHUNKS = [(0, 64), (64, PH * E_LOC - 64)]
        FINISHES = {0: [(0, 0, 64)], 1: [(0, 64, E_LOC - 64), (1, 0, E_LOC)]}
        alu_min = mybir.AluOpType.min
        alu_max = mybir.AluOpType.max
        ident = mybir.ActivationFunctionType.Identity

        def side_ap(tile, off, dims):
            base = tile[:]
            return AP(
                tensor=base.tensor,
                offset=base.offset + off,
                ap=[list(base.ap[0])] + [[s, c] for (s, c) in dims],
            )

        otile = out_p.tile([128, PH * E_LOC], f32, name="otile")
        red_t = out_p.tile([128, PH * E_LOC], f32, name="red_t")

        def emit_sort(goff, GE, first_tt):
            co = goff * DEG
            stages = _sort_stages(GE)
            cur, oth = B, C
            for lo_off, lo_dims, hi_off, hi_dims in stages:
                for op, w_off, w_dims in (
                    (alu_min, lo_off, lo_dims),
                    (alu_max, hi_off, hi_dims),
                ):
                    tt = nc.vector.tensor_tensor(
                        out=side_ap(oth, co + w_off, w_dims),
                        in0=side_ap(cur, co + lo_off, lo_dims),
                        in1=side_ap(cur, co + hi_off, hi_dims),
                        op=op,
                    )
                    if first_tt:
                        _add_dep_helper(
                            tt.ins, dve_gate.ins, sync=False,
                            reason="order sort after DVE consts gate",
                        )
                        first_tt = False
                cur, oth = oth, cur
            return cur, oth

        def emit_finish(h, eoff, ECE, cur, oth):
            """DVE A-multiply + pairwise-add tree + reduce-8; ACT affine."""
            co = (h * E_LOC + eoff) * DEG
            a_h = a_sb[:, h * DEG:(h + 1) * DEG].unsqueeze(1).broadcast_to(
                [128, ECE, DEG]
            )
            nc.vector.tensor_tensor(
                out=side_ap(oth, co, [(DEG, ECE), (1, DEG)]),
                in0=side_ap(cur, co, [(DEG, ECE), (1, DEG)]),
                in1=a_h,
                op=mybir.AluOpType.mult,
            )
            # pairwise-add levels stay in the DVE 2x mode; the final 1x
            # tensor_reduce then only sees 4 terms per edge.
            for w in (16, 8, 4):
                lo = side_ap(oth, co, [(DEG, ECE), (1, w)])
                nc.vector.tensor_tensor(
                    out=lo, in0=lo,
                    in1=side_ap(oth, co + w, [(DEG, ECE), (1, w)]),
                    op=mybir.AluOpType.add,
                )
            rsl = red_t[:, h * E_LOC + eoff:h * E_LOC + eoff + ECE]
            nc.vector.tensor_reduce(
                out=rsl, in_=side_ap(oth, co, [(DEG, ECE), (1, 4)]),
                axis=mybir.AxisListType.X, op=mybir.AluOpType.add,
            )
            osl = otile[:, h * E_LOC + eoff:h * E_LOC + eoff + ECE]
            nc.scalar.activation(
                osl, rsl, ident,
                bias=c_sb[:, h:h + 1], scale=-1.0 / M_ANCH,
            )
            if eoff + ECE == E_LOC:
                nc.sync.dma_start(
                    o_d[h], otile[:, h * E_LOC:(h + 1) * E_LOC])

        for ci, (goff, GE) in enumerate(SORT_CHUNKS):
            cur, oth = emit_sort(goff, GE, first_tt=(ci == 0))
            for h, eoff, ECE in FINISHES[ci]:
                emit_finish(h, eoff, ECE, cur, oth)


def _build():
    import concourse.bacc as bacc
    import concourse.mybir as mybir
    import concourse.tile as tile

    nc = bacc.Bacc(
        "TRN2", target_bir_lowering=False, debug=False,
        enable_asserts=False, num_devices=N_CORES,
    )
    f32 = mybir.dt.float32
    in_d = nc.dram_tensor("xtc", [128, IN_W], f32, kind="ExternalInput").ap()
    o_d = nc.dram_tensor(
        "o", [PH, 128, E_LOC], f32, kind="ExternalOutput"
    ).ap()
    with tile.TileContext(nc) as tc:
        _emit(tc, in_d, o_d)
    nc.compile()
    return nc


_CACHE = {}


def _host_consts(theta_v, weight, anchors):
    import ml_dtypes

    W = theta_v / np.linalg.norm(theta_v, axis=1, keepdims=True)
    u = np.linspace(0.0, 1.0, M_ANCH, dtype=np.float32) * np.float32(0.99998)
    a = u * np.float32(DEG - 1.0) / np.float32(0.99999)
    r0 = np.clip(np.floor(a), 0.0, DEG - 2.0)
    frac = (a - r0).astype(np.float32)
    r0 = r0.astype(np.int64)
    Rind = np.argsort(anchors, axis=0, kind="stable")          # [M, Pd]
    wperm = np.zeros((M_ANCH, N_PROJ), np.float32)
    np.put_along_axis(wperm, Rind, weight.T, axis=0)
    A = np.zeros((DEG, N_PROJ), np.float32)
    np.add.at(A, r0, wperm * (1.0 - frac)[:, None])
    np.add.at(A, r0 + 1, wperm * frac[:, None])
    c = (weight * anchors.T).mean(axis=1).astype(np.float32)   # [Pd]

    # physical slot layout: rank r lives at slot rot(r)
    A_phys = np.zeros_like(A)
    for r in range(DEG):
        A_phys[_rot(r)] = A[r]
    A2 = np.zeros((128, PH * DEG), np.float32)
    c2 = np.zeros((128, PH), np.float32)
    for h in range(PH):
        A2[:, h * DEG:(h + 1) * DEG] = A_phys[:, h * 128:(h + 1) * 128].T
        c2[:, h] = c[h * 128:(h + 1) * 128]
    A2_packed = (
        A2.astype(ml_dtypes.bfloat16).view(np.uint16)
        .reshape(128, PH * DEG).view(np.uint32).view(np.float32)
    )                                                          # [128, 32]
    Wt_packed = (
        np.ascontiguousarray(W.T).astype(ml_dtypes.bfloat16).view(np.uint16)
        .reshape(128, N_PROJ).view(np.uint32).view(np.float32)
    )                                                          # [128, 128]
    consts = np.zeros((128, CONST_W), np.float32)
    consts[:, :W_COLS] = Wt_packed
    consts[:, W_COLS:W_COLS + AB_COLS] = A2_packed
    consts[:, CONST_W - PH:CONST_W] = c2
    return consts


def kernel(X, hyperedge_index, theta_v, weight, anchors, num_edges):
    global LAST_RESULTS
    import ml_dtypes
    from concourse.bass_utils import run_bass_kernel_spmd

    X = np.asarray(X, dtype=np.float32)
    theta_v = np.asarray(theta_v, dtype=np.float32)
    weight = np.asarray(weight, dtype=np.float32)
    anchors = np.asarray(anchors, dtype=np.float32)

    consts = _host_consts(theta_v, weight, anchors)
    XT = np.ascontiguousarray(X.T)                             # [128, N]
    XTb = XT.astype(ml_dtypes.bfloat16).view(np.uint16)        # [128, N] u16
    if "nc" not in _CACHE:
        _CACHE["nc"] = _build()
    nc = _CACHE["nc"]

    in_maps = []
    for cid in range(N_CORES):
        xtc = np.empty((128, IN_W), np.float32)
        xtc[:, :CONST_W] = consts
        xtc[:, CONST_W:] = (
            XTb[:, cid * ROWS_LOC:(cid + 1) * ROWS_LOC]
            .reshape(128, X_COLS, 2).view(np.uint32)[..., 0].view(np.float32)
        )
        in_maps.append({"xtc": xtc})
    res = run_bass_kernel_spmd(
        nc, in_maps, core_ids=list(range(N_CORES)),
        trace=bool(int(os.environ.get("KERNEL_TRACE", "0"))),
    )
    LAST_RESULTS = res

    outT = np.empty((N_PROJ, E_EDGES), np.float32)
    for cid in range(N_CORES):
        o = res.results[cid]["o"]                    # [PH, 128, E_LOC]
        outT[:, cid * E_LOC:(cid + 1) * E_LOC] = o.reshape(N_PROJ, E_LOC)
    return np.ascontiguousarray(outT.T)
